# revision 35
# baseline (speedup 1.0000x reference)
"""Trainium2 Bass kernel for nn_AttentionHead_5583457485447 (sparse_attention).

Reference computation (per batch b):
    q = X @ Wq; k = X @ Wk                      # [N, DK]
    s = relu((q @ k.T) / sqrt(DK)) * M_mask     # [N, N]
    out = s @ Z @ Wv                            # [N, DV]

Strategy (8 NeuronCores, data-parallel over batch B=8, one batch per core):
  - Fold 1/sqrt(DK) into Wv (relu is positively homogeneous, rest is linear).
  - Mask quantized to uint8 (m8 = round(m*255)); 1/255 folded into Wv too.
    Halves mask HBM traffic; adds ~2e-3 rel error (budget 2e-2).
  - ZW = Z @ (Wv/(8*255)) on device; the v=256 contraction sliver (rank-1
    term z256 (x) wv256) rides as a 258th zw column, so the C matmul
    accumulates u[n] = masked8 @ z256' for free and a fused DVE
    scalar_tensor_tensor adds u[p]*wvb + psum during the PSUM->SBUF copy.
  - q/k projections run twice with swapped PE column groups, yielding
    [qT; kT] and [kT; qT] stacked tiles so the score matmuls can row-pack
    two K=64 m-chunks per PE pass (row groups 0:64 / 64:128) with
    LDWEIGHTS pulled ahead across row groups.
  - Scores computed directly in transposed [m, n] layout (lhsT = kT,
    rhs = qT): already the lhsT layout the second matmul needs.
  - relu+mask fused at mt-pair granularity ([128,1024] ops over 2-bank
    psums): DVE scalar_tensor_tensor max(s,0)*m8 for most pairs; ACT relu
    + GpSimd multiply for the rest.
  - DMA trigger count minimized (each dma_start costs its issuing engine
    ~650ns of sequencer time): mask pre-tiled on host into 4 transfers of
    [128, 8192] u8 (8KB lines), XT/Wq/Wk/Wv2 host-packed into single
    transfers. Bulk triggers on sync; xt/w on gpsimd; compute engines
    carry no DMA triggers.
  - Quarter-granularity pipeline: C(q) overlaps B(q+2) scores+elementwise.
"""

import json
import os
import sys

import numpy as np

B, N, D, DK = 8, 2048, 256, 64
DV = D + 1  # 257
NT = N // 128  # 16 tiles along n and along m
PW = 512  # quarter width = scores matmul moving width
NQ = N // PW  # 4 quarters
QT = PW // 128  # 4 n-tiles per quarter

LAST_EXEC_NS = None
_CACHE = {}


# --------------------------------------------------------------------------
# Patch: this container's walrus build rejects instructions carrying more
# than one semaphore wait. Split excess waits onto same-engine NOPs at the
# serialized-BIR level (generic, covers Tile's drains and compute ops).
# --------------------------------------------------------------------------
def _split_waits_in_bir(bir_json: bytes) -> bytes:
    bir = json.loads(bir_json)
    changed = False
    for fn in bir.get("functions", []):
        for bb in fn.get("blocks", []):
            insts = bb.get("instructions", [])
            out = []
            for inst in insts:
                si = inst.get("sync_info")
                ow = (si or {}).get("on_wait") or []
                if len(ow) > 1:
                    changed = True
                    for i, w in enumerate(ow[:-1]):
                        out.append({
                            "debug": inst.get("debug", 0),
                            "engine": inst["engine"],
                            "ins": [],
                            "name": f"{inst['name']}-ws{i}",
                            "opcode": "NoOp",
                            "outs": [],
                            "sync_info": {"on_update": [], "on_wait": [w]},
                            "text_hint": "wait_split",
                        })
                    si["on_wait"] = [ow[-1]]
                out.append(inst)
            bb["instructions"] = out
    return json.dumps(bir).encode() if changed else bir_json


def _apply_bir_patch():
    import concourse.bass_utils as bass_utils
    import concourse.bass2jax as bass2jax

    orig = bass_utils.compile_bir_kernel
    if getattr(orig, "_wait_split_wrapped", False):
        return

    def wrapped(bir_json, tmpdir, neff_name="file.neff"):
        if isinstance(bir_json, str):
            bir_json = bir_json.encode()
        return orig(_split_waits_in_bir(bir_json), tmpdir, neff_name=neff_name)

    wrapped._wait_split_wrapped = True
    bass_utils.compile_bir_kernel = wrapped
    bass2jax.compile_bir_kernel = wrapped


# --------------------------------------------------------------------------
# Optional NTFF profiling hook for axon (exec-time measurement).
# Only used when KERNEL_TRACE=1; missing in this image's antenv.
# --------------------------------------------------------------------------
def _install_profile_shim():
    import types, ctypes, contextlib

    if "antenv.axon_hooks" in sys.modules:
        return
    so_path = "/opt/axon/libaxon_pjrt.so"
    if not os.path.exists(so_path):
        return
    lib = ctypes.CDLL(so_path)
    if not hasattr(lib, "axon_start_nrt_profile"):
        return
    lib.axon_start_nrt_profile.argtypes = [ctypes.POINTER(ctypes.c_int64), ctypes.c_size_t]
    lib.axon_start_nrt_profile.restype = ctypes.c_int64
    lib.axon_stop_nrt_profile.argtypes = [ctypes.c_char_p]
    lib.axon_stop_nrt_profile.restype = ctypes.c_int64

    @contextlib.contextmanager
    def _hook(output_dir, device_ids):
        import jax

        jax.devices()
        if device_ids:
            ids = (ctypes.c_int64 * len(device_ids))(*device_ids)
            rc = lib.axon_start_nrt_profile(ids, len(device_ids))
        else:
            rc = lib.axon_start_nrt_profile(None, 0)
        if rc != 0:
            raise RuntimeError(f"axon_start_nrt_profile rc={rc}")
        try:
            yield
        finally:
            n = lib.axon_stop_nrt_profile(str(output_dir).encode())
            print(f"profile: {n} file(s) written to {output_dir}", file=sys.stderr)

    mod = types.ModuleType("antenv.axon_hooks")
    mod.get_axon_ntff_profile_hook = lambda: _hook
    sys.modules["antenv.axon_hooks"] = mod


# --------------------------------------------------------------------------
# Device program (identical for all 8 cores; one batch per core)
# --------------------------------------------------------------------------
def _build_nc():
    import concourse.bass as bass
    import concourse.mybir as mybir
    import concourse.tile as tile

    f32 = mybir.dt.float32
    bf16 = mybir.dt.bfloat16
    u8 = mybir.dt.uint8
    Alu = mybir.AluOpType
    Act = mybir.ActivationFunctionType

    nc = bass.Bass("TRN2", debug=False)

    # m8p[q*128+p, mt*512+j] = mask[mt*128+p, q*512+j]  (host pre-tiled)
    d_m8 = nc.dram_tensor("m8p", [NQ * 128, NT * PW], u8, kind="ExternalInput")
    # XTp[p, c*2048+n] = X[n, c*128+p]  (both c-chunks side by side)
    d_XT = nc.dram_tensor("XTp", [128, 2 * N], bf16, kind="ExternalInput")
    d_ZT = nc.dram_tensor("ZT", [D, N], bf16, kind="ExternalInput")  # rows 0:256 of Z^T
    d_z256 = nc.dram_tensor("z256", [128, NT], bf16, kind="ExternalInput")
    d_wvb = nc.dram_tensor("wvb", [128, DV], bf16, kind="ExternalInput")
    # wqk[p, :] = [Wq c0 | Wq c1 | Wk c0 | Wk c1]
    d_wqk = nc.dram_tensor("wqk", [128, 4 * DK], bf16, kind="ExternalInput")
    d_Wv2 = nc.dram_tensor("Wv2p", [128, 2 * DV], bf16, kind="ExternalInput")
    d_out = nc.dram_tensor("out", [N, DV], f32, kind="ExternalOutput")

    with tile.TileContext(nc) as tc:
        with (
            tc.tile_pool(name="prep", bufs=1) as prep,       # XT/ZT staging
            tc.tile_pool(name="wts", bufs=1) as wts,         # weights + QK tiles
            tc.tile_pool(name="maskp", bufs=1) as maskp,     # 4 mask tiles resident
            tc.tile_pool(name="maskedp", bufs=1) as maskedp, # masked pair tiles resident
            tc.tile_pool(name="outp", bufs=3) as outp,       # out staging
            tc.tile_pool(name="rlp", bufs=3) as rlp,         # relu staging (ACT path)
            tc.tile_pool(name="zwp", bufs=1) as zwp,         # bf16 ZW pair tiles
            tc.tile_pool(name="psS", bufs=2, space="PSUM") as psS,   # proj/score pairs (2 banks each)
            tc.tile_pool(name="psZ", bufs=1, space="PSUM") as psZ,   # zw pairs (2 banks)
            tc.tile_pool(name="psC", bufs=2, space="PSUM") as psC,   # C groups
        ):
            # ---- sync: wqk + xt quarters first (critical path), then
            # masks q0, ZT, small weights, masks q1-3 ----
            wqk_sb = wts.tile([128, 4 * DK], bf16, tag="wqk", name="wqk")
            nc.sync.dma_start(wqk_sb[:], d_wqk.ap()[:, :])
            # xt quarter tiles: [128, 1024] = [c0-slice | c1-slice] per g
            xtq = [prep.tile([128, 2 * PW], bf16, tag=f"xtq{g}", name=f"xtq{g}")
                   for g in range(NQ)]
            for g in range(NQ):
                nc.sync.dma_start(xtq[g][:], d_XT.ap()[:, g * 2 * PW:(g + 1) * 2 * PW])

            mk = [maskp.tile([128, NT * PW], u8, tag=f"mk{q}", name=f"mk{q}")
                  for q in range(NQ)]

            def emit_mask_q(q):
                nc.sync.dma_start(mk[q][:], d_m8.ap()[q * 128:(q + 1) * 128, :])

            zt_sb = [prep.tile([128, N], bf16, tag=f"zt{i}", name=f"zt{i}") for i in range(2)]
            for i in range(2):
                nc.sync.dma_start(zt_sb[i][:], d_ZT.ap()[i * 128:(i + 1) * 128, :])
            emit_mask_q(0)
            wv2_sb = wts.tile([128, 2 * DV], bf16, tag="wv2", name="wv2")
            nc.sync.dma_start(wv2_sb[:], d_Wv2.ap()[:, :])
            z256_sb = wts.tile([128, NT], bf16, tag="z256", name="z256")
            nc.sync.dma_start(z256_sb[:], d_z256.ap()[:, :])
            wvb_sb = wts.tile([128, DV], bf16, tag="wvb", name="wvb")
            nc.sync.dma_start(wvb_sb[:], d_wvb.ap()[:, :])
            for q in range(1, NQ):
                emit_mask_q(q)

            # Dummy activation triggers the one-time ACT_TABLE_LOAD (~1.3us)
            # before ACT is on the critical path.
            actwu = wts.tile([128, 1], bf16, tag="actwu", name="actwu")
            nc.scalar.activation(actwu[:], wqk_sb[:, 0:1], Act.Relu)

            # ---- projections, twice with swapped column groups ----
            # (cold at first; they double as the HAM warm-up)
            # qk_a[g] = [qT_g (rows 0:64); kT_g (rows 64:128)]
            # qk_b[g] = [kT_g (rows 0:64); qT_g (rows 64:128)]
            qk_a = [wts.tile([128, PW], bf16, tag=f"qka{g}", name=f"qka{g}") for g in range(NQ)]
            qk_b = [wts.tile([128, PW], bf16, tag=f"qkb{g}", name=f"qkb{g}") for g in range(NQ)]
            WQ, WK = 0, 2 * DK  # offsets in wqk_sb

            def emit_proj(g):
                xs = [xtq[g][:, c * PW:(c + 1) * PW] for c in range(2)]
                pa = psS.tile([128, 2 * PW], f32, tag="psS", name=f"psqa{g}")
                for c in range(2):
                    nc.tensor.matmul(pa[0:DK, 0:PW], wqk_sb[:, WQ + c * DK:WQ + (c + 1) * DK],
                                     xs[c], start=(c == 0), stop=(c == 1))
                for c in range(2):
                    nc.tensor.matmul(pa[DK:128, 0:PW], wqk_sb[:, WK + c * DK:WK + (c + 1) * DK],
                                     xs[c], start=(c == 0), stop=(c == 1),
                                     tile_position=(0, DK))
                nc.scalar.activation(qk_a[g][:], pa[:, 0:PW], Act.Copy)
                pb = psS.tile([128, 2 * PW], f32, tag="psS", name=f"psqb{g}")
                for c in range(2):
                    nc.tensor.matmul(pb[0:DK, 0:PW], wqk_sb[:, WK + c * DK:WK + (c + 1) * DK],
                                     xs[c], start=(c == 0), stop=(c == 1))
                for c in range(2):
                    nc.tensor.matmul(pb[DK:128, 0:PW], wqk_sb[:, WQ + c * DK:WQ + (c + 1) * DK],
                                     xs[c], start=(c == 0), stop=(c == 1),
                                     tile_position=(0, DK))
                nc.vector.tensor_copy(qk_b[g][:], pb[:, 0:PW])

            # ---- ZW = Z[:, :256] @ Wv2 in mt-pairs, z256/255 as col 257 ----
            zw_pair = [None] * (NT // 2)

            def emit_zw_pair(pr):
                ps = psZ.tile([128, 1024], f32, tag="psZ", name=f"pszw{pr}")
                for j in range(2):
                    mt = 2 * pr + j
                    for i in range(2):
                        nc.tensor.matmul(
                            ps[:, j * 512:j * 512 + DV],
                            zt_sb[i][:, mt * 128:(mt + 1) * 128],
                            wv2_sb[:, i * DV:(i + 1) * DV],
                            start=(i == 0), stop=(i == 1),
                        )
                zw = zwp.tile([128, 2 * (DV + 1)], bf16, tag=f"zw{pr}", name=f"zw{pr}")
                zw_v = zw[:].rearrange("p (j v) -> p j v", j=2)[:, :, 0:DV]
                ps_v = ps[:].rearrange("p (j v) -> p j v", j=2)[:, :, 0:DV]
                nc.scalar.activation(zw_v, ps_v, Act.Copy)
                for j in range(2):
                    mt = 2 * pr + j
                    nc.vector.tensor_copy(
                        zw[:, j * (DV + 1) + DV:j * (DV + 1) + DV + 1],
                        z256_sb[:, mt:mt + 1],
                    )
                zw_pair[pr] = zw

            def zw_rhs(mt):
                return zw_pair[mt // 2][:, (mt % 2) * (DV + 1):(mt % 2) * (DV + 1) + DV + 1]

            # ---- B(q): scores + relu*mask at mt-pair granularity ----
            masked_sb = {}
            ew = 0  # elementwise rotation counter

            def emit_b_pair(q, pr):
                nonlocal ew
                ps = psS.tile([128, 2 * PW], f32, tag="psS", name=f"pss{q}_{pr}")
                for j in range(2):
                    mt = 2 * pr + j
                    gk, kcol = divmod(mt * 128, PW)
                    ro = DK * j
                    # j=0: lhsT = kT from qk_b rows 0:64; rhs = qT from qk_a rows 0:64
                    # j=1: lhsT = kT from qk_a rows 64:128; rhs = qT from qk_b rows 64:128
                    lhsT = (qk_b if j == 0 else qk_a)[gk][ro:ro + DK, kcol:kcol + 128]
                    rhs = (qk_a if j == 0 else qk_b)[q][ro:ro + DK, :]
                    nc.tensor.matmul(ps[:, j * PW:(j + 1) * PW], lhsT, rhs,
                                     start=True, stop=True)
                md = maskedp.tile([128, 2 * PW], bf16, tag=f"md{q}_{pr}", name=f"md{q}_{pr}")
                masked_sb[(q, pr)] = md
                mkap = mk[q][:, 2 * pr * PW:2 * (pr + 1) * PW]
                if ew % 8 in (2, 5, 7):
                    rl = rlp.tile([128, 2 * PW], bf16, tag="rl", name=f"rl{q}_{pr}")
                    nc.scalar.activation(rl[:], ps[:], Act.Relu)
                    nc.gpsimd.tensor_mul(md[:], rl[:], mkap)
                else:
                    nc.vector.scalar_tensor_tensor(
                        md[:], ps[:], 0.0, mkap, Alu.max, Alu.mult,
                    )
                ew += 1

            def masked_ap(q, mt, i):
                # [128,128] slice of the masked pair tile for C's lhsT
                return masked_sb[(q, mt // 2)][:, (mt % 2) * PW + i * 128:(mt % 2) * PW + (i + 1) * 128]

            # ---- C(q, nt): out n-tile = sum_mt masked.T @ zw ----
            def emit_c_group(q, i):
                nt = q * QT + i
                ps = psC.tile([128, DV + 1], f32, tag="psC", name=f"psc{nt}")
                for mt in range(NT):
                    nc.tensor.matmul(
                        ps[:],
                        masked_ap(q, mt, i),
                        zw_rhs(mt),
                        start=(mt == 0), stop=(mt == NT - 1),
                    )
                ot = outp.tile([128, DV], f32, tag="out", name=f"ot{nt}")
                # out = wvb * u + main   (u = psum col 257)
                nc.vector.scalar_tensor_tensor(
                    ot[:], wvb_sb[:], ps[:, DV:DV + 1], ps[:, 0:DV],
                    Alu.mult, Alu.add,
                )
                nc.sync.dma_start(
                    d_out.ap()[nt * 128:(nt + 1) * 128, :], ot[:]
                )

            # ---- software pipeline over quarters ----
            # [proj x B(0)] -> [ZW x B(1)] -> [C(0) x B(2)] -> [C(1) x B(3)]
            # -> C(2) -> C(3).  B(0) pairs 2g,2g+1 depend exactly on proj g
            # (plus proj 0 for the rhs), so they interleave into proj's
            # xt-arrival gaps.
            for g in range(NQ):
                emit_proj(g)
                emit_b_pair(0, 2 * g)
                emit_b_pair(0, 2 * g + 1)
            for pr in range(NT // 2):
                emit_zw_pair(pr)
                emit_b_pair(1, pr)
            for q in (2, 3):
                for pr in range(NT // 2):
                    if pr % 2 == 0:
                        emit_c_group(q - 2, pr // 2)
                    emit_b_pair(q, pr)
            for q in (2, 3):
                for i in range(QT):
                    emit_c_group(q, i)

    return nc


def kernel(Z_l, X_l, M_mask, Wq, Wk, Wv):
    global LAST_EXEC_NS
    _apply_bir_patch()

    trace = os.environ.get("KERNEL_TRACE", "0") == "1"
    if trace:
        _install_profile_shim()

    from concourse.bass_utils import run_bass_kernel_spmd

    Z_l = np.asarray(Z_l, dtype=np.float32)
    X_l = np.asarray(X_l, dtype=np.float32)
    M_mask = np.asarray(M_mask, dtype=np.float32)
    Wq = np.asarray(Wq, dtype=np.float32)
    Wk = np.asarray(Wk, dtype=np.float32)
    Wv = np.asarray(Wv, dtype=np.float32)

    import ml_dtypes
    bf = ml_dtypes.bfloat16

    # Host-side layout prep (transpose + casts) + scale folds.
    scale = np.float32(1.0 / (np.sqrt(np.float32(DK)) * 255.0))
    XT = np.ascontiguousarray(X_l.transpose(0, 2, 1)).astype(bf)          # [B, D, N]
    # quarter-major packing: XTp[b, p, g*1024 + c*512 + j] = XT[b, c*128+p, g*512+j]
    XTp = np.ascontiguousarray(
        XT.reshape(B, 2, 128, NQ, PW).transpose(0, 2, 3, 1, 4).reshape(B, 128, 2 * N)
    )
    ZT = np.ascontiguousarray(Z_l[:, :, :D].transpose(0, 2, 1)).astype(bf)  # [B, 256, N]
    # mask -> u8, pre-tiled: m8p[b, q*128+p, mt*512+j] = m8T[b, mt*128+p, q*512+j]
    M8T = np.clip(np.round(M_mask * 255.0), 0, 255).astype(np.uint8).transpose(0, 2, 1)
    M8P = np.ascontiguousarray(
        M8T.reshape(B, NT, 128, NQ, PW).transpose(0, 3, 2, 1, 4).reshape(B, NQ * 128, NT * PW)
    )
    z256 = np.ascontiguousarray(
        (Z_l[:, :, D] / np.float32(255.0)).reshape(B, NT, 128).transpose(0, 2, 1)
    ).astype(bf)                                                           # [B, 128, 16]
    wvb = np.ascontiguousarray(
        np.broadcast_to(Wv[D, :] / np.sqrt(np.float32(DK)), (128, DV))
    ).astype(bf)                                                           # [128, 257]
    # packed weights: [128, 4*DK] = [Wq c0 | Wq c1 | Wk c0 | Wk c1]
    wqk = np.concatenate([
        Wq.reshape(2, 128, DK).transpose(1, 0, 2).reshape(128, 2 * DK),
        Wk.reshape(2, 128, DK).transpose(1, 0, 2).reshape(128, 2 * DK),
    ], axis=1).astype(bf)
    Wv2p = np.ascontiguousarray(
        (Wv[:D, :] * scale).reshape(2, 128, DV).transpose(1, 0, 2).reshape(128, 2 * DV)
    ).astype(bf)

    if "nc" not in _CACHE:
        _CACHE["nc"] = _build_nc()
    nc = _CACHE["nc"]

    in_maps = [
        {
            "m8p": M8P[b],
            "XTp": XTp[b],
            "ZT": ZT[b],
            "z256": z256[b],
            "wvb": wvb,
            "wqk": wqk,
            "Wv2p": Wv2p,
        }
        for b in range(B)
    ]
    try:
        res = run_bass_kernel_spmd(nc, in_maps, core_ids=list(range(B)), trace=trace)
    except Exception:
        # A prior (profiled) run can leave an execution unit wedged; the failed
        # attempt clears it and a retry goes through.
        res = run_bass_kernel_spmd(nc, in_maps, core_ids=list(range(B)), trace=trace)
    _CACHE["last_res"] = res
    if trace:
        LAST_EXEC_NS = res.exec_time_ns
    out = np.stack([res.results[b]["out"] for b in range(B)], axis=0)
    return out


# revision 36
# speedup vs baseline: 1.1768x; 1.1768x over previous
"""Trainium2 Bass kernel for nn_AttentionHead_5583457485447 (sparse_attention).

Reference computation (per batch b):
    q = X @ Wq; k = X @ Wk                      # [N, DK]
    s = relu((q @ k.T) / sqrt(DK)) * M_mask     # [N, N]
    out = s @ Z @ Wv                            # [N, DV]

Strategy (8 NeuronCores, data-parallel over batch B=8, one batch per core):
  - Fold 1/sqrt(DK) into Wv (relu is positively homogeneous, rest is linear).
  - Mask quantized to uint8 (m8 = round(m*255)); 1/255 folded into Wv too.
    Halves mask HBM traffic; adds ~2e-3 rel error (budget 2e-2).
  - ZW = Z @ (Wv/(8*255)) on device; the v=256 contraction sliver (rank-1
    term z256 (x) wv256) rides as a 258th zw column, so the C matmul
    accumulates u[n] = masked8 @ z256' for free and a fused DVE
    scalar_tensor_tensor adds u[p]*wvb + psum during the PSUM->SBUF copy.
  - q/k projections run twice with swapped PE column groups, yielding
    [qT; kT] and [kT; qT] stacked tiles so the score matmuls can row-pack
    two K=64 m-chunks per PE pass (row groups 0:64 / 64:128) with
    LDWEIGHTS pulled ahead across row groups.
  - Scores computed directly in transposed [m, n] layout (lhsT = kT,
    rhs = qT): already the lhsT layout the second matmul needs.
  - relu+mask fused at mt-pair granularity ([128,1024] ops over 2-bank
    psums): DVE scalar_tensor_tensor max(s,0)*m8 for most pairs; ACT relu
    + GpSimd multiply for the rest.
  - DMA trigger count minimized (each dma_start costs its issuing engine
    ~650ns of sequencer time): mask pre-tiled on host into 4 transfers of
    [128, 8192] u8 (8KB lines), XT/Wq/Wk/Wv2 host-packed into single
    transfers. Bulk triggers on sync; xt/w on gpsimd; compute engines
    carry no DMA triggers.
  - Quarter-granularity pipeline: C(q) overlaps B(q+2) scores+elementwise.
"""

import json
import os
import sys

import numpy as np

B, N, D, DK = 8, 2048, 256, 64
DV = D + 1  # 257
NT = N // 128  # 16 tiles along n and along m
PW = 512  # quarter width = scores matmul moving width
NQ = N // PW  # 4 quarters
QT = PW // 128  # 4 n-tiles per quarter

LAST_EXEC_NS = None
_CACHE = {}


# --------------------------------------------------------------------------
# Patch: this container's walrus build rejects instructions carrying more
# than one semaphore wait. Split excess waits onto same-engine NOPs at the
# serialized-BIR level (generic, covers Tile's drains and compute ops).
# --------------------------------------------------------------------------
def _split_waits_in_bir(bir_json: bytes) -> bytes:
    bir = json.loads(bir_json)
    changed = False
    for fn in bir.get("functions", []):
        for bb in fn.get("blocks", []):
            insts = bb.get("instructions", [])
            out = []
            for inst in insts:
                si = inst.get("sync_info")
                ow = (si or {}).get("on_wait") or []
                if len(ow) > 1:
                    changed = True
                    for i, w in enumerate(ow[:-1]):
                        out.append({
                            "debug": inst.get("debug", 0),
                            "engine": inst["engine"],
                            "ins": [],
                            "name": f"{inst['name']}-ws{i}",
                            "opcode": "NoOp",
                            "outs": [],
                            "sync_info": {"on_update": [], "on_wait": [w]},
                            "text_hint": "wait_split",
                        })
                    si["on_wait"] = [ow[-1]]
                out.append(inst)
            bb["instructions"] = out
    return json.dumps(bir).encode() if changed else bir_json


def _apply_bir_patch():
    import concourse.bass_utils as bass_utils
    import concourse.bass2jax as bass2jax

    orig = bass_utils.compile_bir_kernel
    if getattr(orig, "_wait_split_wrapped", False):
        return

    def wrapped(bir_json, tmpdir, neff_name="file.neff"):
        if isinstance(bir_json, str):
            bir_json = bir_json.encode()
        return orig(_split_waits_in_bir(bir_json), tmpdir, neff_name=neff_name)

    wrapped._wait_split_wrapped = True
    bass_utils.compile_bir_kernel = wrapped
    bass2jax.compile_bir_kernel = wrapped


# --------------------------------------------------------------------------
# Optional NTFF profiling hook for axon (exec-time measurement).
# Only used when KERNEL_TRACE=1; missing in this image's antenv.
# --------------------------------------------------------------------------
def _install_profile_shim():
    import types, ctypes, contextlib

    if "antenv.axon_hooks" in sys.modules:
        return
    so_path = "/opt/axon/libaxon_pjrt.so"
    if not os.path.exists(so_path):
        return
    lib = ctypes.CDLL(so_path)
    if not hasattr(lib, "axon_start_nrt_profile"):
        return
    lib.axon_start_nrt_profile.argtypes = [ctypes.POINTER(ctypes.c_int64), ctypes.c_size_t]
    lib.axon_start_nrt_profile.restype = ctypes.c_int64
    lib.axon_stop_nrt_profile.argtypes = [ctypes.c_char_p]
    lib.axon_stop_nrt_profile.restype = ctypes.c_int64

    @contextlib.contextmanager
    def _hook(output_dir, device_ids):
        import jax

        jax.devices()
        if device_ids:
            ids = (ctypes.c_int64 * len(device_ids))(*device_ids)
            rc = lib.axon_start_nrt_profile(ids, len(device_ids))
        else:
            rc = lib.axon_start_nrt_profile(None, 0)
        if rc != 0:
            raise RuntimeError(f"axon_start_nrt_profile rc={rc}")
        try:
            yield
        finally:
            n = lib.axon_stop_nrt_profile(str(output_dir).encode())
            print(f"profile: {n} file(s) written to {output_dir}", file=sys.stderr)

    mod = types.ModuleType("antenv.axon_hooks")
    mod.get_axon_ntff_profile_hook = lambda: _hook
    sys.modules["antenv.axon_hooks"] = mod


# --------------------------------------------------------------------------
# Device program (identical for all 8 cores; one batch per core)
# --------------------------------------------------------------------------
def _build_nc():
    import concourse.bass as bass
    import concourse.mybir as mybir
    import concourse.tile as tile

    f32 = mybir.dt.float32
    bf16 = mybir.dt.bfloat16
    u8 = mybir.dt.uint8
    Alu = mybir.AluOpType
    Act = mybir.ActivationFunctionType

    nc = bass.Bass("TRN2", debug=False)

    # m8p[q*128+p, mt*512+j] = mask[mt*128+p, q*512+j]  (host pre-tiled)
    d_m8 = nc.dram_tensor("m8p", [NQ * 128, NT * PW], u8, kind="ExternalInput")
    # XTp[p, c*2048+n] = X[n, c*128+p]  (both c-chunks side by side)
    d_XT = nc.dram_tensor("XTp", [128, 2 * N], bf16, kind="ExternalInput")
    d_ZT = nc.dram_tensor("ZT", [D, N], bf16, kind="ExternalInput")  # rows 0:256 of Z^T
    d_z256 = nc.dram_tensor("z256", [128, NT], bf16, kind="ExternalInput")
    d_wvb = nc.dram_tensor("wvb", [128, DV], bf16, kind="ExternalInput")
    # wqk[p, :] = [Wq c0 | Wq c1 | Wk c0 | Wk c1]
    d_wqk = nc.dram_tensor("wqk", [128, 4 * DK], bf16, kind="ExternalInput")
    d_Wv2 = nc.dram_tensor("Wv2p", [128, 2 * DV], bf16, kind="ExternalInput")
    d_out = nc.dram_tensor("out", [N, DV], f32, kind="ExternalOutput")

    with tile.TileContext(nc) as tc:
        with (
            tc.tile_pool(name="prep", bufs=1) as prep,       # XT/ZT staging
            tc.tile_pool(name="wts", bufs=1) as wts,         # weights + QK tiles
            tc.tile_pool(name="maskp", bufs=1) as maskp,     # 4 mask tiles resident
            tc.tile_pool(name="maskedp", bufs=1) as maskedp, # masked pair tiles resident
            tc.tile_pool(name="outp", bufs=3) as outp,       # out staging
            tc.tile_pool(name="rlp", bufs=3) as rlp,         # relu staging (ACT path)
            tc.tile_pool(name="zwp", bufs=1) as zwp,         # bf16 ZW pair tiles
            tc.tile_pool(name="psS", bufs=2, space="PSUM") as psS,   # proj/score pairs (2 banks each)
            tc.tile_pool(name="psZ", bufs=1, space="PSUM") as psZ,   # zw pairs (2 banks)
            tc.tile_pool(name="psC", bufs=2, space="PSUM") as psC,   # C groups
        ):
            # ---- sync: wqk + xt quarters first (critical path), then
            # masks q0, ZT, small weights, masks q1-3 ----
            wqk_sb = wts.tile([128, 4 * DK], bf16, tag="wqk", name="wqk")
            nc.sync.dma_start(wqk_sb[:], d_wqk.ap()[:, :])
            # xt quarter tiles: [128, 1024] = [c0-slice | c1-slice] per g
            xtq = [prep.tile([128, 2 * PW], bf16, tag=f"xtq{g}", name=f"xtq{g}")
                   for g in range(NQ)]
            for g in range(NQ):
                nc.sync.dma_start(xtq[g][:], d_XT.ap()[:, g * 2 * PW:(g + 1) * 2 * PW])

            mk = [maskp.tile([128, NT * PW], u8, tag=f"mk{q}", name=f"mk{q}")
                  for q in range(NQ)]

            def emit_mask_q(q):
                nc.sync.dma_start(mk[q][:], d_m8.ap()[q * 128:(q + 1) * 128, :])

            emit_mask_q(0)
            zt_sb = [prep.tile([128, N], bf16, tag=f"zt{i}", name=f"zt{i}") for i in range(2)]
            for i in range(2):
                nc.sync.dma_start(zt_sb[i][:], d_ZT.ap()[i * 128:(i + 1) * 128, :])
            wv2_sb = wts.tile([128, 2 * DV], bf16, tag="wv2", name="wv2")
            nc.sync.dma_start(wv2_sb[:], d_Wv2.ap()[:, :])
            z256_sb = wts.tile([128, NT], bf16, tag="z256", name="z256")
            nc.sync.dma_start(z256_sb[:], d_z256.ap()[:, :])
            wvb_sb = wts.tile([128, DV], bf16, tag="wvb", name="wvb")
            nc.sync.dma_start(wvb_sb[:], d_wvb.ap()[:, :])
            for q in range(1, NQ):
                emit_mask_q(q)

            # Dummy activation triggers the one-time ACT_TABLE_LOAD (~1.3us)
            # before ACT is on the critical path.
            actwu = wts.tile([128, 1], bf16, tag="actwu", name="actwu")
            nc.scalar.activation(actwu[:], wqk_sb[:, 0:1], Act.Relu)

            # ---- projections, twice with swapped column groups ----
            # (cold at first; they double as the HAM warm-up)
            # qk_a[g] = [qT_g (rows 0:64); kT_g (rows 64:128)]
            # qk_b[g] = [kT_g (rows 0:64); qT_g (rows 64:128)]
            qk_a = [wts.tile([128, PW], bf16, tag=f"qka{g}", name=f"qka{g}") for g in range(NQ)]
            qk_b = [wts.tile([128, PW], bf16, tag=f"qkb{g}", name=f"qkb{g}") for g in range(NQ)]
            WQ, WK = 0, 2 * DK  # offsets in wqk_sb

            def emit_proj(g):
                xs = [xtq[g][:, c * PW:(c + 1) * PW] for c in range(2)]
                pa = psS.tile([128, 2 * PW], f32, tag="psS", name=f"psqa{g}")
                for c in range(2):
                    nc.tensor.matmul(pa[0:DK, 0:PW], wqk_sb[:, WQ + c * DK:WQ + (c + 1) * DK],
                                     xs[c], start=(c == 0), stop=(c == 1))
                for c in range(2):
                    nc.tensor.matmul(pa[DK:128, 0:PW], wqk_sb[:, WK + c * DK:WK + (c + 1) * DK],
                                     xs[c], start=(c == 0), stop=(c == 1),
                                     tile_position=(0, DK))
                nc.scalar.activation(qk_a[g][:], pa[:, 0:PW], Act.Copy)
                pb = psS.tile([128, 2 * PW], f32, tag="psS", name=f"psqb{g}")
                for c in range(2):
                    nc.tensor.matmul(pb[0:DK, 0:PW], wqk_sb[:, WK + c * DK:WK + (c + 1) * DK],
                                     xs[c], start=(c == 0), stop=(c == 1))
                for c in range(2):
                    nc.tensor.matmul(pb[DK:128, 0:PW], wqk_sb[:, WQ + c * DK:WQ + (c + 1) * DK],
                                     xs[c], start=(c == 0), stop=(c == 1),
                                     tile_position=(0, DK))
                nc.vector.tensor_copy(qk_b[g][:], pb[:, 0:PW])

            # ---- ZW = Z[:, :256] @ Wv2 in mt-pairs, z256/255 as col 257 ----
            zw_pair = [None] * (NT // 2)

            def emit_zw_pair(pr):
                ps = psZ.tile([128, 1024], f32, tag="psZ", name=f"pszw{pr}")
                for j in range(2):
                    mt = 2 * pr + j
                    for i in range(2):
                        nc.tensor.matmul(
                            ps[:, j * 512:j * 512 + DV],
                            zt_sb[i][:, mt * 128:(mt + 1) * 128],
                            wv2_sb[:, i * DV:(i + 1) * DV],
                            start=(i == 0), stop=(i == 1),
                        )
                zw = zwp.tile([128, 2 * (DV + 1)], bf16, tag=f"zw{pr}", name=f"zw{pr}")
                zw_v = zw[:].rearrange("p (j v) -> p j v", j=2)[:, :, 0:DV]
                ps_v = ps[:].rearrange("p (j v) -> p j v", j=2)[:, :, 0:DV]
                nc.scalar.activation(zw_v, ps_v, Act.Copy)
                for j in range(2):
                    mt = 2 * pr + j
                    nc.vector.tensor_copy(
                        zw[:, j * (DV + 1) + DV:j * (DV + 1) + DV + 1],
                        z256_sb[:, mt:mt + 1],
                    )
                zw_pair[pr] = zw

            def zw_rhs(mt):
                return zw_pair[mt // 2][:, (mt % 2) * (DV + 1):(mt % 2) * (DV + 1) + DV + 1]

            # ---- B(q): scores + relu*mask at mt-pair granularity ----
            masked_sb = {}
            ew = 0  # elementwise rotation counter

            def emit_b_pair(q, pr):
                nonlocal ew
                ps = psS.tile([128, 2 * PW], f32, tag="psS", name=f"pss{q}_{pr}")
                for j in range(2):
                    mt = 2 * pr + j
                    gk, kcol = divmod(mt * 128, PW)
                    ro = DK * j
                    # j=0: lhsT = kT from qk_b rows 0:64; rhs = qT from qk_a rows 0:64
                    # j=1: lhsT = kT from qk_a rows 64:128; rhs = qT from qk_b rows 64:128
                    lhsT = (qk_b if j == 0 else qk_a)[gk][ro:ro + DK, kcol:kcol + 128]
                    rhs = (qk_a if j == 0 else qk_b)[q][ro:ro + DK, :]
                    nc.tensor.matmul(ps[:, j * PW:(j + 1) * PW], lhsT, rhs,
                                     start=True, stop=True)
                md = maskedp.tile([128, 2 * PW], bf16, tag=f"md{q}_{pr}", name=f"md{q}_{pr}")
                masked_sb[(q, pr)] = md
                mkap = mk[q][:, 2 * pr * PW:2 * (pr + 1) * PW]
                if ew % 8 in (2, 5, 7):
                    rl = rlp.tile([128, 2 * PW], bf16, tag="rl", name=f"rl{q}_{pr}")
                    nc.scalar.activation(rl[:], ps[:], Act.Relu)
                    nc.gpsimd.tensor_mul(md[:], rl[:], mkap)
                else:
                    nc.vector.scalar_tensor_tensor(
                        md[:], ps[:], 0.0, mkap, Alu.max, Alu.mult,
                    )
                ew += 1

            def masked_ap(q, mt, i):
                # [128,128] slice of the masked pair tile for C's lhsT
                return masked_sb[(q, mt // 2)][:, (mt % 2) * PW + i * 128:(mt % 2) * PW + (i + 1) * 128]

            # ---- C(q, nt): out n-tile = sum_mt masked.T @ zw ----
            def emit_c_group(q, i):
                nt = q * QT + i
                ps = psC.tile([128, DV + 1], f32, tag="psC", name=f"psc{nt}")
                for mt in range(NT):
                    nc.tensor.matmul(
                        ps[:],
                        masked_ap(q, mt, i),
                        zw_rhs(mt),
                        start=(mt == 0), stop=(mt == NT - 1),
                    )
                ot = outp.tile([128, DV], f32, tag="out", name=f"ot{nt}")
                # out = wvb * u + main   (u = psum col 257)
                nc.vector.scalar_tensor_tensor(
                    ot[:], wvb_sb[:], ps[:, DV:DV + 1], ps[:, 0:DV],
                    Alu.mult, Alu.add,
                )
                nc.sync.dma_start(
                    d_out.ap()[nt * 128:(nt + 1) * 128, :], ot[:]
                )

            # ---- software pipeline over quarters ----
            # [proj x B(0)] -> [ZW x B(1)] -> [C(0) x B(2)] -> [C(1) x B(3)]
            # -> C(2) -> C(3).  B(0) pairs 2g,2g+1 depend exactly on proj g
            # (plus proj 0 for the rhs), so they interleave into proj's
            # xt-arrival gaps.
            for g in range(NQ):
                emit_proj(g)
                emit_b_pair(0, 2 * g)
                emit_b_pair(0, 2 * g + 1)
            for pr in range(NT // 2):
                emit_zw_pair(pr)
                emit_b_pair(1, pr)
            for q in (2, 3):
                for pr in range(NT // 2):
                    if pr % 2 == 0:
                        emit_c_group(q - 2, pr // 2)
                    emit_b_pair(q, pr)
            for q in (2, 3):
                for i in range(QT):
                    emit_c_group(q, i)

    return nc


def kernel(Z_l, X_l, M_mask, Wq, Wk, Wv):
    global LAST_EXEC_NS
    _apply_bir_patch()

    trace = os.environ.get("KERNEL_TRACE", "0") == "1"
    if trace:
        _install_profile_shim()

    from concourse.bass_utils import run_bass_kernel_spmd

    Z_l = np.asarray(Z_l, dtype=np.float32)
    X_l = np.asarray(X_l, dtype=np.float32)
    M_mask = np.asarray(M_mask, dtype=np.float32)
    Wq = np.asarray(Wq, dtype=np.float32)
    Wk = np.asarray(Wk, dtype=np.float32)
    Wv = np.asarray(Wv, dtype=np.float32)

    import ml_dtypes
    bf = ml_dtypes.bfloat16

    # Host-side layout prep (transpose + casts) + scale folds.
    scale = np.float32(1.0 / (np.sqrt(np.float32(DK)) * 255.0))
    XT = np.ascontiguousarray(X_l.transpose(0, 2, 1)).astype(bf)          # [B, D, N]
    # quarter-major packing: XTp[b, p, g*1024 + c*512 + j] = XT[b, c*128+p, g*512+j]
    XTp = np.ascontiguousarray(
        XT.reshape(B, 2, 128, NQ, PW).transpose(0, 2, 3, 1, 4).reshape(B, 128, 2 * N)
    )
    ZT = np.ascontiguousarray(Z_l[:, :, :D].transpose(0, 2, 1)).astype(bf)  # [B, 256, N]
    # mask -> u8, pre-tiled: m8p[b, q*128+p, mt*512+j] = m8T[b, mt*128+p, q*512+j]
    M8T = np.clip(np.round(M_mask * 255.0), 0, 255).astype(np.uint8).transpose(0, 2, 1)
    M8P = np.ascontiguousarray(
        M8T.reshape(B, NT, 128, NQ, PW).transpose(0, 3, 2, 1, 4).reshape(B, NQ * 128, NT * PW)
    )
    z256 = np.ascontiguousarray(
        (Z_l[:, :, D] / np.float32(255.0)).reshape(B, NT, 128).transpose(0, 2, 1)
    ).astype(bf)                                                           # [B, 128, 16]
    wvb = np.ascontiguousarray(
        np.broadcast_to(Wv[D, :] / np.sqrt(np.float32(DK)), (128, DV))
    ).astype(bf)                                                           # [128, 257]
    # packed weights: [128, 4*DK] = [Wq c0 | Wq c1 | Wk c0 | Wk c1]
    wqk = np.concatenate([
        Wq.reshape(2, 128, DK).transpose(1, 0, 2).reshape(128, 2 * DK),
        Wk.reshape(2, 128, DK).transpose(1, 0, 2).reshape(128, 2 * DK),
    ], axis=1).astype(bf)
    Wv2p = np.ascontiguousarray(
        (Wv[:D, :] * scale).reshape(2, 128, DV).transpose(1, 0, 2).reshape(128, 2 * DV)
    ).astype(bf)

    if "nc" not in _CACHE:
        _CACHE["nc"] = _build_nc()
    nc = _CACHE["nc"]

    in_maps = [
        {
            "m8p": M8P[b],
            "XTp": XTp[b],
            "ZT": ZT[b],
            "z256": z256[b],
            "wvb": wvb,
            "wqk": wqk,
            "Wv2p": Wv2p,
        }
        for b in range(B)
    ]
    try:
        res = run_bass_kernel_spmd(nc, in_maps, core_ids=list(range(B)), trace=trace)
    except Exception:
        # A prior (profiled) run can leave an execution unit wedged; the failed
        # attempt clears it and a retry goes through.
        res = run_bass_kernel_spmd(nc, in_maps, core_ids=list(range(B)), trace=trace)
    _CACHE["last_res"] = res
    if trace:
        LAST_EXEC_NS = res.exec_time_ns
    out = np.stack([res.results[b]["out"] for b in range(B)], axis=0)
    return out
